# revision 2
# baseline (speedup 1.0000x reference)
"""ATTConv (GNN bilinear-attention message passing) on 8 Trainium2 NeuronCores.

Strategy (self-contained: full inputs in, full output out):
  * Host sorts edges by dst and shards them by dst-node range across the 8
    cores (12500 nodes/core); all segment reductions are core-local.
  * Algebraic refold: a_e = theta(x_s).phi(x_d) = x_s . q_d with
    q = (W_theta^T W_phi) x; qT ([feat, node], fp16) is computed on-device
    in phase 0 and kept SBUF-resident, so no per-edge q gather exists.
  * Nodes are grouped in 128-node windows (98/core), windows in supertiles
    of 4.  Edges are grouped by (window, src-quarter) and padded to
    128-edge chunks (chunk counts equalized across cores -> SPMD).
  * One SWDGE dma_gather per (supertile, quarter) fetches x[src] rows
    (fp16, [edge, feat] layout).  Per chunk, TensorE transposes the rows
    ([feat, edge], via PE transpose + ACT copy) and computes all-pairs
    scores S = X^T_chunk q_win^T into PSUM.  A second accumulating matmul
    adds (BIG/scale)*onehot(dst_rel) using a host-streamed fp8 one-hot
    tile (sequential SWDGE load - no per-edge descriptors), and one ACT
    exp(scale*S - BIG) produces the masked edge weights SpT[e, slot]
    directly - the main loop issues no DVE work at all, which matters
    because DVE activity stalls SWDGE gather descriptor processing (~3x
    its own duration in measured gather slowdown).
  * Aggregation: G^T[f, slot] accumulates X_chunk^T @ SpT per window in
    PSUM; softmax denominators are never computed - LayerNorm is
    invariant to per-row scaling, so z = G^T W_fc^T is normalized
    directly (the eps shift is ~1e-5/den^2, negligible).
  * Epilogue per supertile: 4 Z matmuls, ACT-side sum/sum-of-squares
    (accum_out) for LN stats, ACT Identity(scale=rstd, bias=-mu*rstd),
    gamma/beta on DVE, one output DMA per window.
"""
import os
import sys

for _p in ("/opt/trn_rl_repo", "/root/.axon_site/_ro/trn_rl_repo"):
    if os.path.isdir(_p):
        if _p not in sys.path:
            sys.path.insert(0, _p)
        break

import numpy as np

import concourse.bass as bass
import concourse.bacc as bacc
import concourse.tile as tile
from concourse import mybir
from concourse.bass_utils import run_bass_kernel_spmd

P = 128
N = 100000
E = 1600000
D = 128
NCORE = 8
NLOC = N // NCORE
NWIN = (NLOC + P - 1) // P
NPAD = NWIN * P
NQUART = 4
QROWS = N // NQUART
SUPER = 4
SCALE = float(D) ** -0.5
LN_EPS = 1e-5
F32 = mybir.dt.float32
F16 = mybir.dt.float16
I16 = mybir.dt.int16
PAD_REL = 1000.0
BIG = 30.0


def _supertiles(nwin):
    return [list(range(s, min(s + SUPER, nwin))) for s in range(0, nwin, SUPER)]


def _wrap16(idx_flat):
    s = len(idx_flat) // 16
    base = idx_flat.reshape(s, 16).T.astype(np.int16)
    return np.tile(base, (8, 1))


def build_kernel(cwr, nloc=NLOC, nwin=NWIN, qrows=QROWS, passes=1):
    npad = nwin * P
    nchunks = int(sum(sum(r) for r in cwr))
    nedge = nchunks * P

    nc = bacc.Bacc("TRN2", target_bir_lowering=False, debug=False,
                   enable_asserts=False, num_devices=NCORE,
                   num_swdge_queues=4)

    xq = [nc.dram_tensor(f"x_q{r}", [qrows, P], F16, kind="ExternalInput")
          for r in range(NQUART)]
    xlT = nc.dram_tensor("xlocT", [P, npad], F16, kind="ExternalInput")
    xidx = nc.dram_tensor("xidx_w", [P, nedge // 16], I16, kind="ExternalInput")
    ohd = nc.dram_tensor("oh_p", [P, nchunks * P], mybir.dt.float8e4,
                         kind="ExternalInput")
    mqT = nc.dram_tensor("mqT", [P, P], F16, kind="ExternalInput")
    wfcT = nc.dram_tensor("wfcT", [P, P], F32, kind="ExternalInput")
    gbc = nc.dram_tensor("gamma_bc", [P, P], F32, kind="ExternalInput")
    bbc = nc.dram_tensor("beta_bc", [P, P], F32, kind="ExternalInput")
    zout = nc.dram_tensor("zout", [nloc, P], F32, kind="ExternalOutput")

    iota_f = np.tile(np.arange(P, dtype=np.float16)[None, :], (P, 1))
    iota_f_d = nc.inline_tensor(iota_f, "iota_f")
    ident_f = np.eye(P, dtype=np.float16)
    ident_d = nc.inline_tensor(ident_f, "ident_f")
    bigi_f = (np.eye(P) * (BIG / SCALE)).astype(np.float16)
    bigi_d = nc.inline_tensor(bigi_f, "bigi_f")

    # static chunk schedule
    sts = _supertiles(nwin)
    chunk_of = []
    x_gathers = []
    st_span = []          # (st_first_chunk, n_st)
    for si, ws in enumerate(sts):
        st_first = len(chunk_of)
        for r in range(NQUART):
            n_idx = sum(cwr[w][r] for w in ws) * P
            if n_idx:
                x_gathers.append((si, r, n_idx, len(chunk_of)))
            for w in ws:
                for _ in range(cwr[w][r]):
                    chunk_of.append((w, r, si))
        st_span.append((st_first, (len(chunk_of) - st_first) * P))
    assert len(chunk_of) == nchunks
    group_first = {}
    for ci, (w, r, si) in enumerate(chunk_of):
        group_first.setdefault((w, r), ci)


    with tile.TileContext(nc) as tc:
        with (
            tc.tile_pool(name="const", bufs=1) as cpool,
            tc.tile_pool(name="psz", bufs=2, space="PSUM") as psz,
            tc.tile_pool(name="psg", bufs=2, space="PSUM") as psg,
            tc.tile_pool(name="pss", bufs=2, space="PSUM") as pss,
        ):
            IF = cpool.tile([P, P], F16, tag="if")
            nc.sync.dma_start(IF[:], iota_f_d.ap()[:, :])
            MQ = cpool.tile([P, P], F16, tag="mq")
            nc.sync.dma_start(MQ[:], mqT.ap()[:, :])
            WT = cpool.tile([P, P], F32, tag="wt")
            nc.sync.dma_start(WT[:], wfcT.ap()[:, :])
            GB = cpool.tile([P, P], F32, tag="gb")
            nc.sync.dma_start(GB[:], gbc.ap()[:, :])
            BB = cpool.tile([P, P], F32, tag="bb")
            nc.sync.dma_start(BB[:], bbc.ap()[:, :])
            eps_col = cpool.tile([P, 1], F32, tag="eps_c")
            nc.vector.memset(eps_col[:], LN_EPS)
            nbig_col = cpool.tile([P, 1], F32, tag="nbig_c")
            nc.vector.memset(nbig_col[:], -BIG)
            ID16 = cpool.tile([P, P], F16, tag="id16")
            nc.sync.dma_start(ID16[:], ident_d.ap()[:, :])
            BIGI = cpool.tile([P, P], F16, tag="bigi")
            nc.sync.dma_start(BIGI[:], bigi_d.ap()[:, :])

            # ---- phase 0: qT_all[f', node] resident in SBUF (fp16) ----
            qT_all = cpool.tile([P, npad], F16, tag="qt")
            with tc.tile_pool(name="ph0", bufs=1) as p0:
                xlT_sb = p0.tile([P, npad], F16, tag="xlt")
                nc.sync.dma_start(xlT_sb[:], xlT.ap()[:, :])
                QB = 512
                for b in range(npad // QB):
                    q_ps = psz.tile([P, QB], F32, tag="z", name=f"q0_{b}")
                    nc.tensor.matmul(out=q_ps[:], lhsT=MQ[:],
                                     rhs=xlT_sb[:, b*QB:(b+1)*QB],
                                     start=True, stop=True)
                    nc.scalar.activation(qT_all[:, b*QB:(b+1)*QB], q_ps[:],
                                         mybir.ActivationFunctionType.Copy)
                rem = npad - (npad // QB) * QB
                if rem:
                    b0 = (npad // QB) * QB
                    q_ps = psz.tile([P, QB], F32, tag="z", name="q0_rem")
                    nc.tensor.matmul(out=q_ps[:, :rem], lhsT=MQ[:],
                                     rhs=xlT_sb[:, b0:npad],
                                     start=True, stop=True)
                    nc.scalar.activation(qT_all[:, b0:npad], q_ps[:, :rem],
                                         mybir.ActivationFunctionType.Copy)

            # ---- main loop ----
            with (
                tc.tile_pool(name="idx", bufs=2) as ipool,
                tc.tile_pool(name="gx", bufs=8) as gxpool,
                tc.tile_pool(name="oh", bufs=2) as ohpool,
                tc.tile_pool(name="wk", bufs=4) as wpool,
                tc.tile_pool(name="ep", bufs=2) as epool,
            ):
                for pass_ in range(passes):
                  for si, ws in enumerate(sts):
                    st_x = [g for g in x_gathers if g[0] == si]
                    st_first, n_st = st_span[si]
                    nch_st = n_st // P

                    xcols = n_st // 16
                    xi_sb = ipool.tile([P, xcols], I16, tag="xi")
                    nc.sync.dma_start(
                        xi_sb[:], xidx.ap()[:, st_first*8:st_first*8 + xcols])
                    OH_sb = ohpool.tile([P, nch_st * P], F16, tag="oh",
                                        name=f"oh_{si}")
                    nc.gpsimd.dma_start(
                        OH_sb[:], ohd.ap()[:, st_first*P:st_first*P + n_st])

                    X_st = {}
                    for gi, (_, r, n_idx, fc) in enumerate(st_x):
                        xt_sb = gxpool.tile([P, (n_idx // P) * P], F16,
                                            tag="xg")
                        off = (fc - st_first) * 8
                        nc.gpsimd.dma_gather(
                            out_ap=xt_sb[:].rearrange("p (b e) -> p b e", e=P),
                            in_ap=xq[r].ap()[:, :],
                            idxs_ap=xi_sb[:, off:off + n_idx // 16],
                            num_idxs=n_idx, num_idxs_reg=n_idx, elem_size=P,
                            single_packet=False, queue_num=r)
                        X_st[r] = (xt_sb, fc)

                    # ---- per window: scores -> exp -> one-hot -> aggregate
                    Gt_all = psg.tile([P, SUPER * P], F32, tag="g",
                                      name=f"gt_{si}")
                    MAXC = 24
                    for wl, w in enumerate(ws):
                        wchunks = [(r, k) for r in range(NQUART)
                                   for k in range(cwr[w][r])]
                        nwc = len(wchunks)
                        assert nwc <= MAXC
                        qT_win = qT_all[:, w*P:(w+1)*P]
                        SpT_w = wpool.tile([P, MAXC * P], F16, tag="spw")
                        for b0 in range(0, nwc, 4):
                            bts = wchunks[b0:b0+4]
                            nb = len(bts)
                            xT_ps = pss.tile([P, 4 * P], F16, tag="xtp")
                            for bi, (r, k) in enumerate(bts):
                                ci = group_first[(w, r)] + k
                                xt_sb, fc = X_st[r]
                                xb = ci - fc
                                nc.tensor.transpose(
                                    xT_ps[:, bi*P:(bi+1)*P],
                                    xt_sb[:, xb*P:(xb+1)*P], ID16[:])
                            xT_sb = wpool.tile([P, 4 * P], F16, tag="xts")
                            nc.scalar.activation(
                                xT_sb[:, :nb*P], xT_ps[:, :nb*P],
                                mybir.ActivationFunctionType.Copy)
                            S_ps = pss.tile([P, 4 * P], F32, tag="s")
                            for bi, (r, k) in enumerate(bts):
                                ci = group_first[(w, r)] + k
                                nc.tensor.matmul(
                                    out=S_ps[:, bi*P:(bi+1)*P],
                                    lhsT=xT_sb[:, bi*P:(bi+1)*P],
                                    rhs=qT_win,
                                    start=True, stop=False)
                                lci = ci - st_first
                                nc.tensor.matmul(
                                    out=S_ps[:, bi*P:(bi+1)*P],
                                    lhsT=BIGI[:],
                                    rhs=OH_sb[:, lci*P:(lci+1)*P],
                                    start=False, stop=True)
                            nc.scalar.activation(
                                SpT_w[:, b0*P:b0*P + nb*P], S_ps[:, :nb*P],
                                mybir.ActivationFunctionType.Exp,
                                scale=SCALE, bias=nbig_col[:, :1])
                        for ki, (r, k) in enumerate(wchunks):
                            ci = group_first[(w, r)] + k
                            xt_sb, fc = X_st[r]
                            xb = ci - fc
                            nc.tensor.matmul(
                                out=Gt_all[:, wl*P:(wl+1)*P],
                                lhsT=xt_sb[:, xb*P:(xb+1)*P],
                                rhs=SpT_w[:, ki*P:(ki+1)*P],
                                start=(ki == 0), stop=(ki == nwc - 1))

                    # ---- epilogue (same as stage-1) ----
                    nws = len(ws)
                    Gt_sb = epool.tile([P, SUPER * P], F32, tag="gt")
                    nc.scalar.activation(Gt_sb[:, :nws*P], Gt_all[:, :nws*P],
                                         mybir.ActivationFunctionType.Copy)
                    Z_all = psz.tile([P, SUPER * P], F32, tag="z",
                                     name=f"z_{si}")
                    for wl in range(nws):
                        nc.tensor.matmul(out=Z_all[:, wl*P:(wl+1)*P],
                                         lhsT=Gt_sb[:, wl*P:(wl+1)*P],
                                         rhs=WT[:], start=True, stop=True)
                    s1 = epool.tile([P, SUPER], F32, tag="s1")
                    nc.vector.tensor_reduce(
                        out=s1[:, :nws],
                        in_=Z_all[:, :nws*P].rearrange("p (w f) -> p w f", f=P),
                        axis=mybir.AxisListType.X,
                        op=mybir.AluOpType.add)
                    s2 = epool.tile([P, SUPER], F32, tag="s2")
                    zsq = epool.tile([P, P], F32, tag="zsq")
                    for wl in range(nws):
                        nc.scalar.activation(
                            zsq[:], Z_all[:, wl*P:(wl+1)*P],
                            mybir.ActivationFunctionType.Square,
                            accum_out=s2[:, wl:wl+1])
                    sq1 = epool.tile([P, SUPER], F32, tag="sq1")
                    nc.vector.tensor_tensor(out=sq1[:, :nws], in0=s1[:, :nws],
                                            in1=s1[:, :nws],
                                            op=mybir.AluOpType.mult)
                    d2 = epool.tile([P, SUPER], F32, tag="d2")
                    nc.vector.tensor_scalar(
                        out=d2[:, :nws], in0=sq1[:, :nws],
                        scalar1=-1.0 / P, scalar2=None,
                        op0=mybir.AluOpType.mult)
                    vv = epool.tile([P, SUPER], F32, tag="vv")
                    nc.vector.tensor_tensor(out=vv[:, :nws], in0=s2[:, :nws],
                                            in1=d2[:, :nws],
                                            op=mybir.AluOpType.add)
                    std = epool.tile([P, SUPER], F32, tag="std")
                    nc.scalar.activation(
                        std[:, :nws], vv[:, :nws],
                        mybir.ActivationFunctionType.Sqrt,
                        scale=1.0 / P, bias=eps_col[:, :1])
                    rstd = epool.tile([P, SUPER], F32, tag="rstd")
                    nc.vector.reciprocal(rstd[:, :nws], std[:, :nws])
                    nmu = epool.tile([P, SUPER], F32, tag="nmu")
                    nc.vector.tensor_tensor(
                        out=nmu[:, :nws], in1=rstd[:, :nws], in0=s1[:, :nws],
                        op=mybir.AluOpType.mult)
                    nmun = epool.tile([P, SUPER], F32, tag="nmun")
                    nc.vector.tensor_scalar(
                        out=nmun[:, :nws], in0=nmu[:, :nws],
                        scalar1=-1.0 / P, scalar2=None,
                        op0=mybir.AluOpType.mult)
                    zn = epool.tile([P, SUPER * P], F32, tag="zn")
                    for wl in range(nws):
                        nc.scalar.activation(
                            zn[:, wl*P:(wl+1)*P], Z_all[:, wl*P:(wl+1)*P],
                            mybir.ActivationFunctionType.Identity,
                            scale=rstd[:, wl:wl+1],
                            bias=nmun[:, wl:wl+1])
                    ob = epool.tile([P, SUPER * P], F32, tag="ob")
                    for wl in range(nws):
                        og = epool.tile([P, P], F32, tag="og")
                        nc.vector.tensor_tensor(
                            out=og[:], in0=zn[:, wl*P:(wl+1)*P], in1=GB[:],
                            op=mybir.AluOpType.mult)
                        nc.vector.tensor_tensor(
                            out=ob[:, wl*P:(wl+1)*P], in0=og[:], in1=BB[:],
                            op=mybir.AluOpType.add)
                    for wl, w in enumerate(ws):
                        nw_rows = min(P, nloc - w * P)
                        nc.sync.dma_start(
                            zout.ap()[w*P:w*P + nw_rows, :],
                            ob[:nw_rows, wl*P:(wl+1)*P])
    nc.compile()
    return nc


def preprocess(x, W_fc, W_theta, W_phi, gamma, beta, src, dst,
               n=N, ncore=NCORE, nloc=NLOC, nwin=NWIN, qrows=QROWS):
    src = np.ascontiguousarray(src, dtype=np.int64)
    dst = np.ascontiguousarray(dst, dtype=np.int64)
    order = np.argsort(dst, kind="stable")
    src_s = src[order]
    dst_s = dst[order]
    bounds = np.searchsorted(dst_s, np.arange(0, n + 1, nloc))

    per_core = []
    ngroups = nwin * NQUART
    counts = np.zeros((ncore, ngroups), np.int64)
    for d in range(ncore):
        lo, hi = bounds[d], bounds[d + 1]
        dl = dst_s[lo:hi] - d * nloc
        sl = src_s[lo:hi]
        w = dl >> 7
        r = sl // qrows
        g = w * NQUART + r
        ord2 = np.argsort(g, kind="stable")
        per_core.append((dl[ord2], sl[ord2], g[ord2]))
        counts[d] = np.bincount(g, minlength=ngroups)

    cwr_flat = np.ceil(counts.max(axis=0) / P).astype(np.int64)
    cwr = cwr_flat.reshape(nwin, NQUART)
    assert (cwr.sum(axis=1) > 0).all()

    sts = _supertiles(nwin)
    group_slot_off = np.zeros(ngroups, np.int64)
    nslots = 0
    for ws in sts:
        for r in range(NQUART):
            for w in ws:
                g = w * NQUART + r
                group_slot_off[g] = nslots
                nslots += cwr[w][r] * P
    nchunks = nslots // P

    MqT = (W_phi.astype(np.float64).T @ W_theta.astype(np.float64)).astype(np.float16)
    WfcT = np.ascontiguousarray(W_fc.T, dtype=np.float32)
    GBC = np.tile(np.asarray(gamma, np.float32)[None, :], (P, 1))
    BBC = np.tile(np.asarray(beta, np.float32)[None, :], (P, 1))
    x = np.ascontiguousarray(x, dtype=np.float32)
    x16 = x.astype(np.float16)
    xq = {f"x_q{r}": np.ascontiguousarray(x16[r*qrows:(r+1)*qrows])
          for r in range(NQUART)}

    in_maps = []
    for d in range(ncore):
        dl, sl, g = per_core[d]
        j = np.arange(len(dl), dtype=np.int64)
        gstart = np.zeros(ngroups, np.int64)
        cnt = np.bincount(g, minlength=ngroups)
        gstart[1:] = np.cumsum(cnt)[:-1]
        slot = group_slot_off[g] + (j - gstart[g])

        xidx_f = np.zeros(nslots, np.int64)
        xidx_f[slot] = sl % qrows
        dr = np.full(nslots, 128, np.int64)
        dr[slot] = (dl & 127)
        dr2 = dr.reshape(nchunks, P)           # [ci, p]
        ci_i, p_i = np.nonzero(dr2 < 128)
        import ml_dtypes
        oh = np.zeros((P, nchunks * P), ml_dtypes.float8_e4m3)
        oh[p_i, ci_i * P + dr2[ci_i, p_i]] = 1.0

        npadl = nwin * P
        xlocT = np.zeros((P, npadl), np.float16)
        xlocT[:, :nloc] = x16[d * nloc:(d + 1) * nloc].T

        in_maps.append({
            **xq,
            "xlocT": xlocT,
            "xidx_w": _wrap16(xidx_f),
            "oh_p": oh,
            "mqT": MqT,
            "wfcT": WfcT,
            "gamma_bc": GBC,
            "beta_bc": BBC,
        })
    return in_maps, cwr


_cache = {}


def kernel(x, W_fc, W_theta, W_phi, gamma, beta, src, dst):
    in_maps, cwr = preprocess(np.asarray(x), np.asarray(W_fc),
                              np.asarray(W_theta), np.asarray(W_phi),
                              np.asarray(gamma), np.asarray(beta),
                              np.asarray(src), np.asarray(dst))
    key = cwr.tobytes()
    if key not in _cache:
        _cache[key] = build_kernel([list(map(int, row)) for row in cwr])
    nc = _cache[key]
    res = run_bass_kernel_spmd(nc, in_maps, core_ids=list(range(NCORE)))
    out = np.concatenate([res.results[c]["zout"] for c in range(NCORE)], axis=0)
    return np.ascontiguousarray(out, dtype=np.float32)
